# revision 12
# baseline (speedup 1.0000x reference)
"""CrossPath kernel for 8 TRN2 NeuronCores.

Sharding: 8 independent work units = 4 batches x 2 cross-attention paths.
Core c = b*2 + path. Each core computes, for its (b, path):
  Q = xq @ q_w.T ; K,V = xkv @ kv_w.T split
  O_h = softmax(Q_h K_h^T * SCALE) V_h        (beta is softmax-invariant)
  y = LayerNorm(relu(O @ p_w.T + p_b)) ; reshape [C,48,48]; bilinear x2
Device layout is feature-major ("T" = [C, N]) so every matmul contracts on
partitions. Softmax is done in the S^T layout (k on partitions) with the
denominator obtained by appending a ones-column to V (row 32 of the AV psum).
"""

import numpy as np
import ml_dtypes

B, N, C = 4, 2304, 256
H, D = 8, 32
SCALE = D ** -0.5
HDS = 48
EPS = 1e-5
P = 128
NT = N // P          # 18 n-tiles
CC = C // P          # 2 channel chunks
QCS = [1024, 1024, 256]   # q chunking for attention


def _build_nc():
    import concourse.bass as bass
    import concourse.mybir as mybir
    import concourse.tile as tile
    from concourse.masks import make_identity

    # ---- walrus workaround: split multi-wait exit drain ---------------
    from bass_rust import ScopedClock

    def _patched_drain_and_barrier(self, tick_clock, wait_clock):
        nc = self.nc
        probe = nc.sync.nop(nofuse=True)
        wait_clock.add_sem_waits(
            probe.ins, ScopedClock({None: tick_clock.global_clock}))
        si = probe.ins.sync_info
        waits = list(si.on_wait) if si is not None else []
        probe.ins.sync_info = None
        name_to_sem = {s.name: s for s in self.sems.allocated().values()}
        for w in waits:
            nc.sync.wait_ge(name_to_sem[w.ant_name], w.wait_value)
        nc.sync.drain()
        nc.all_engine_barrier()
        popped = nc._tile_sem_poison_stack.pop()
        assert popped is self._sem_poison
        nc.clear_and_free_semaphores(list(self.sems.allocated().values()))
        nc.all_engine_barrier()

    tile.TileContext._drain_and_barrier = _patched_drain_and_barrier

    # Split >MAXW sem waits per instruction onto preceding same-engine NOPs
    # (this walrus codegen rejects instructions with too many sync waits).
    from bass_rust import SyncInfo
    MAXW = 1
    if not getattr(tile.TileContext, "_wait_split_patched", False):
        _orig_lower = tile.TileContext._lower_ordered_insts

        def _split_lower(self, ordered):
            nc = self.nc
            for bb_name, insts in ordered.items():
                out_l = []
                for inst in insts:
                    si = inst.sync_info
                    waits = list(si.on_wait) if si is not None else []
                    if len(waits) > MAXW:
                        extra, keep = waits[:-MAXW], waits[-MAXW:]
                        for i in range(0, len(extra), MAXW):
                            nop = mybir.InstNoOp(
                                name=nc.get_next_instruction_name(),
                                ins=[], outs=[], engine=inst.engine)
                            nop.sync_info = SyncInfo(
                                on_wait=extra[i:i + MAXW], on_update=[])
                            out_l.append(nop)
                        inst.sync_info = SyncInfo(
                            on_wait=keep, on_update=list(si.on_update))
                    out_l.append(inst)
                insts[:] = out_l
            return _orig_lower(self, ordered)

        tile.TileContext._lower_ordered_insts = _split_lower
        tile.TileContext._wait_split_patched = True
    # -------------------------------------------------------------------

    f32 = mybir.dt.float32
    bf16 = mybir.dt.bfloat16
    AF = mybir.ActivationFunctionType

    nc = bass.Bass()
    xqT = nc.dram_tensor("xqT", [P, CC, N], bf16, kind="ExternalInput")
    xkvT = nc.dram_tensor("xkvT", [P, CC, N], bf16, kind="ExternalInput")
    wqT = nc.dram_tensor("wqT", [P, CC, C], bf16, kind="ExternalInput")
    wkT = nc.dram_tensor("wkT", [P, CC, C], bf16, kind="ExternalInput")
    wvT = nc.dram_tensor("wvT", [P, CC, C], bf16, kind="ExternalInput")
    wpT = nc.dram_tensor("wpT", [P, CC, C], bf16, kind="ExternalInput")
    pbb = nc.dram_tensor("pbb", [P, C], f32, kind="ExternalInput")
    gbb = nc.dram_tensor("gbb", [P, C], f32, kind="ExternalInput")
    bbb = nc.dram_tensor("bbb", [P, C], f32, kind="ExternalInput")
    out = nc.dram_tensor("out", [P, CC, 96 * 96], f32, kind="ExternalOutput")

    with tile.TileContext(nc) as tc:
        with (
            tc.tile_pool(name="persist", bufs=1) as pp,
            tc.tile_pool(name="work", bufs=2) as wp,
        ):
            # ---- load inputs ----
            xpool = tc.tile_pool(name="xin", bufs=1)
            xp = xpool.__enter__()
            xq_sb = xp.tile([P, CC, N], bf16, name="xq_sb")
            xkv_sb = xp.tile([P, CC, N], bf16, name="xkv_sb")
            nc.sync.dma_start(xq_sb[:], xqT[:])
            nc.sync.dma_start(xkv_sb[:], xkvT[:])
            w_sb = {}
            for name, t in (("wq", wqT), ("wk", wkT), ("wv", wvT), ("wp", wpT)):
                w_sb[name] = pp.tile([P, CC, C], bf16, tag=f"w_{name}",
                                     name=f"w_{name}")
                nc.sync.dma_start(w_sb[name][:], t[:])
            pb_sb = pp.tile([P, C], f32, tag="pb")
            g_sb = pp.tile([P, C], f32, tag="g")
            b_sb = pp.tile([P, C], f32, tag="b")
            nc.sync.dma_start(pb_sb[:], pbb[:])
            nc.sync.dma_start(g_sb[:], gbb[:])
            nc.sync.dma_start(b_sb[:], bbb[:])

            ident = pp.tile([P, P], f32, tag="ident")
            make_identity(nc, ident)
            ones_col = pp.tile([1, 32], f32, tag="ones32")
            nc.vector.memset(ones_col[:], 1.0)

            # persistent activations
            QT = pp.tile([P, CC, N], bf16, tag="QT")
            KT = pp.tile([P, CC, N], bf16, tag="KT")
            Vaug = pp.tile([P, NT, H, 33], bf16, tag="Vaug")
            OT = pp.tile([P, CC, N], bf16, tag="OT")

            nc.vector.memset(Vaug[:, :, :, 32:33], 1.0)

            # ---- QKV projections ----
            with tc.tile_pool(name="psum_p", bufs=4, space="PSUM") as psp:
                # QT / KT : out [c_out 128, n 512]
                nsl = [(i * 512, min(512, N - i * 512)) for i in range((N + 511) // 512)]
                for dst, wname, src in ((QT, "wq", xq_sb), (KT, "wk", xkv_sb)):
                    for cc in range(CC):
                        for n0, nw in nsl:
                            ps = psp.tile([P, 512], f32, tag="ps_qk")
                            for k in range(CC):
                                nc.tensor.matmul(
                                    ps[:, :nw],
                                    lhsT=w_sb[wname][:, k, cc * P:(cc + 1) * P],
                                    rhs=src[:, k, n0:n0 + nw],
                                    start=(k == 0), stop=(k == CC - 1))
                            nc.scalar.activation(
                                dst[:, cc, n0:n0 + nw], ps[:, :nw], AF.Copy)
                # V natural: out [n 128, c 256]
                for nt in range(NT):
                    ps = psp.tile([P, C], f32, tag="ps_v")
                    for k in range(CC):
                        nc.tensor.matmul(
                            ps[:],
                            lhsT=xkv_sb[:, k, nt * P:(nt + 1) * P],
                            rhs=w_sb["wv"][:, k, :],
                            start=(k == 0), stop=(k == CC - 1))
                    nc.vector.tensor_copy(
                        Vaug[:, nt, :, 0:32],
                        ps.rearrange("p (h d) -> p h d", h=H))

            xpool.__exit__(None, None, None)

            # matmul operands must sit at base partition 0/32/64 — copy the
            # strips of heads at offset 96 down to base-0 aux tiles via DMA
            QTaux = pp.tile([32, CC, N], bf16, tag="QTaux")
            KTaux = pp.tile([32, CC, N], bf16, tag="KTaux")
            nc.sync.dma_start(QTaux[:], QT[96:128])
            nc.sync.dma_start(KTaux[:], KT[96:128])

            # ---- attention ----
            with (
                tc.tile_pool(name="psum_s", bufs=2, space="PSUM") as pss,
                tc.tile_pool(name="psum_o", bufs=1, space="PSUM") as pso,
                tc.tile_pool(name="psum_l", bufs=1, space="PSUM") as psl,
                tc.tile_pool(name="et", bufs=2) as etp,
                tc.tile_pool(name="attn_tmp", bufs=3) as atp,
            ):
                for h in range(H):
                    hc, hs = h // 4, (h % 4) * 32
                    if h % 4 == 3:
                        QTh, KTh, os_ = QTaux, KTaux, 0
                    else:
                        QTh, KTh, os_ = QT, KT, hs
                    q0 = 0
                    for qcw in QCS:
                        ET = etp.tile([P, NT, 1024], bf16, tag="ET")
                        for kt in range(NT):
                            ps = pss.tile([P, 1024], f32, tag="ps_s")
                            for j in range(0, qcw, 512):
                                jw = min(512, qcw - j)
                                nc.tensor.matmul(
                                    ps[:, j:j + jw],
                                    lhsT=KTh[os_:os_ + 32, hc, kt * P:(kt + 1) * P],
                                    rhs=QTh[os_:os_ + 32, hc, q0 + j:q0 + j + jw],
                                    start=True, stop=True)
                            nc.scalar.activation(
                                ET[:, kt, 0:qcw], ps[:, 0:qcw], AF.Exp)
                        po = pso.tile([P, 1024], f32, tag="ps_o")
                        for j in range(0, qcw, 512):
                            jw = min(512, qcw - j)
                            for kt in range(NT):
                                nc.tensor.matmul(
                                    po[0:33, j:j + jw],
                                    lhsT=Vaug[:, kt, h, :],
                                    rhs=ET[:, kt, j:j + jw],
                                    start=(kt == 0), stop=(kt == NT - 1))
                        # normalizer: row 32 holds l[q]
                        rl = atp.tile([1, 1024], f32, tag="rl")
                        nc.vector.reciprocal(rl[0:1, 0:qcw], po[32:33, 0:qcw])
                        pl = psl.tile([P, 1024], f32, tag="ps_l")
                        for j in range(0, qcw, 512):
                            jw = min(512, qcw - j)
                            nc.tensor.matmul(
                                pl[0:32, j:j + jw],
                                lhsT=ones_col[:],
                                rhs=rl[0:1, j:j + jw],
                                start=True, stop=True)
                        pls = atp.tile([32, 1024], f32, tag="pls")
                        nc.vector.tensor_copy(pls[:, 0:qcw], pl[0:32, 0:qcw])
                        nc.vector.tensor_mul(
                            OT[hs:hs + 32, hc, q0:q0 + qcw],
                            po[0:32, 0:qcw], pls[:, 0:qcw])
                        q0 += qcw

            # ---- P projection + relu + layernorm ----
            y2 = pp.tile([P, NT, C], f32, tag="y2")
            with (
                tc.tile_pool(name="psum_y", bufs=2, space="PSUM") as psy,
                tc.tile_pool(name="ln_tmp", bufs=4) as lnp,
            ):
                for nt in range(NT):
                    ps = psy.tile([P, C], f32, tag="ps_y")
                    for k in range(CC):
                        nc.tensor.matmul(
                            ps[:],
                            lhsT=OT[:, k, nt * P:(nt + 1) * P],
                            rhs=w_sb["wp"][:, k, :],
                            start=(k == 0), stop=(k == CC - 1))
                    yt = lnp.tile([P, C], f32, tag="yt")
                    nc.vector.tensor_add(yt[:], ps[:], pb_sb[:])
                    nc.vector.tensor_scalar_max(yt[:], yt[:], 0.0)
                    # layernorm over free axis (C)
                    mu = lnp.tile([P, 1], f32, tag="mu")
                    nc.vector.tensor_reduce(
                        out=mu[:], in_=yt[:], op=mybir.AluOpType.add,
                        axis=mybir.AxisListType.X)
                    nc.vector.tensor_scalar_mul(mu[:], mu[:], 1.0 / C)
                    xc = lnp.tile([P, C], f32, tag="xc")
                    nc.vector.tensor_scalar(
                        out=xc[:], in0=yt[:], scalar1=mu[:, 0:1], scalar2=None,
                        op0=mybir.AluOpType.subtract)
                    sq = lnp.tile([P, C], f32, tag="sq")
                    nc.vector.tensor_mul(sq[:], xc[:], xc[:])
                    var = lnp.tile([P, 1], f32, tag="var")
                    nc.vector.tensor_reduce(
                        out=var[:], in_=sq[:], op=mybir.AluOpType.add,
                        axis=mybir.AxisListType.X)
                    nc.vector.tensor_scalar(
                        out=var[:], in0=var[:], scalar1=1.0 / C, scalar2=EPS,
                        op0=mybir.AluOpType.mult, op1=mybir.AluOpType.add)
                    lnv = lnp.tile([P, 1], f32, tag="lnv")
                    nc.scalar.activation(lnv[:], var[:], AF.Ln)
                    rstd = lnp.tile([P, 1], f32, tag="rstd")
                    nc.scalar.activation(rstd[:], lnv[:], AF.Exp, scale=-0.5)
                    nc.vector.tensor_scalar(
                        out=xc[:], in0=xc[:], scalar1=rstd[:, 0:1], scalar2=None,
                        op0=mybir.AluOpType.mult)
                    nc.vector.tensor_mul(xc[:], xc[:], g_sb[:])
                    nc.vector.tensor_add(y2[:, nt, :], xc[:], b_sb[:])

            # ---- transpose y2 [n, c] -> yT [c, n] ----
            yT = pp.tile([P, CC, N], f32, tag="yT")
            with tc.tile_pool(name="psum_t", bufs=4, space="PSUM") as pst:
                for nt in range(NT):
                    for cc in range(CC):
                        pt = pst.tile([P, P], f32, tag="ps_t")
                        nc.tensor.transpose(
                            pt[:], y2[:, nt, cc * P:(cc + 1) * P], ident[:])
                        nc.scalar.activation(
                            yT[:, cc, nt * P:(nt + 1) * P], pt[:], AF.Copy)

            # ---- bilinear x2 upsample (half-pixel), separable ----
            # along axis: out[2j] = .25 a[j-1] + .75 a[j]   (out[0]=a[0])
            #             out[2j+1] = .75 a[j] + .25 a[j+1] (out[95]=a[47])
            with tc.tile_pool(name="ups", bufs=1) as up:
                for cc in range(CC):
                    src = yT[:, cc, :].rearrange("p (h w) -> p h w", h=HDS)
                    z = up.tile([P, HDS, 96], f32, tag="z")   # w-upsampled
                    zv = z[:]
                    # even outputs j>=1
                    t1 = up.tile([P, HDS, 47], f32, tag="t1")
                    nc.vector.tensor_scalar_mul(t1[:], src[:, :, 1:48], 0.75)
                    nc.vector.tensor_scalar_mul(
                        zv[:, :, 2:96:2], src[:, :, 0:47], 0.25)
                    nc.vector.tensor_add(
                        zv[:, :, 2:96:2], zv[:, :, 2:96:2], t1[:])
                    # odd outputs j<=46
                    nc.vector.tensor_scalar_mul(t1[:], src[:, :, 0:47], 0.75)
                    nc.vector.tensor_scalar_mul(
                        zv[:, :, 1:95:2], src[:, :, 1:48], 0.25)
                    nc.vector.tensor_add(
                        zv[:, :, 1:95:2], zv[:, :, 1:95:2], t1[:])
                    # edges
                    nc.vector.tensor_copy(zv[:, :, 0:1], src[:, :, 0:1])
                    nc.vector.tensor_copy(zv[:, :, 95:96], src[:, :, 47:48])
                    # h pass -> out rows
                    u = up.tile([P, 96, 96], f32, tag="u")
                    uv = u[:]
                    t2 = up.tile([P, 47, 96], f32, tag="t2")
                    nc.vector.tensor_scalar_mul(t2[:], zv[:, 1:48, :], 0.75)
                    nc.vector.tensor_scalar_mul(
                        uv[:, 2:96:2, :], zv[:, 0:47, :], 0.25)
                    nc.vector.tensor_add(
                        uv[:, 2:96:2, :], uv[:, 2:96:2, :], t2[:])
                    nc.vector.tensor_scalar_mul(t2[:], zv[:, 0:47, :], 0.75)
                    nc.vector.tensor_scalar_mul(
                        uv[:, 1:95:2, :], zv[:, 1:48, :], 0.25)
                    nc.vector.tensor_add(
                        uv[:, 1:95:2, :], uv[:, 1:95:2, :], t2[:])
                    nc.vector.tensor_copy(uv[:, 0:1, :], zv[:, 0:1, :])
                    nc.vector.tensor_copy(uv[:, 95:96, :], zv[:, 47:48, :])
                    nc.sync.dma_start(
                        out[:, cc, :],
                        u.rearrange("p a b -> p (a b)"))
    return nc


_CACHED = {}


def _get_nc():
    if "nc" not in _CACHED:
        _CACHED["nc"] = _build_nc()
    return _CACHED["nc"]


def _core_inputs(xq, xkv, q_w, kv_w, p_w, p_b, g, b):
    bf = ml_dtypes.bfloat16

    def fm(w):  # [Cin,Cout] -> [P, CC, Cout]
        return np.ascontiguousarray(
            w.reshape(CC, P, -1).transpose(1, 0, 2)).astype(bf)

    return {
        "xqT": fm(np.ascontiguousarray(xq.T)),
        "xkvT": fm(np.ascontiguousarray(xkv.T)),
        "wqT": fm(np.ascontiguousarray((q_w * SCALE).T)),
        "wkT": fm(np.ascontiguousarray(kv_w[:C].T)),
        "wvT": fm(np.ascontiguousarray(kv_w[C:].T)),
        "wpT": fm(np.ascontiguousarray(p_w.T)),
        "pbb": np.ascontiguousarray(np.broadcast_to(p_b, (P, C))).astype(np.float32),
        "gbb": np.ascontiguousarray(np.broadcast_to(g, (P, C))).astype(np.float32),
        "bbb": np.ascontiguousarray(np.broadcast_to(b, (P, C))).astype(np.float32),
    }


def _make_in_maps(x1, x2, q1_w, kv1_w, q2_w, kv2_w,
                  p1_w, p1_b, p2_w, p2_b, g1, b1, g2, b2):
    in_maps = []
    for b in range(B):
        in_maps.append(_core_inputs(
            x1[b], x2[b], q1_w, kv2_w, p1_w, p1_b, g1, b1))
        in_maps.append(_core_inputs(
            x2[b], x1[b], q2_w, kv1_w, p2_w, p2_b, g2, b2))
    return in_maps


def kernel(x1, x2, q1_w, kv1_w, q2_w, kv2_w, beta12, beta21,
           p1_w, p1_b, p2_w, p2_b, g1, b1, g2, b2):
    from concourse.bass_utils import run_bass_kernel_spmd

    x1 = np.asarray(x1, np.float32)
    x2 = np.asarray(x2, np.float32)
    in_maps = _make_in_maps(
        np.asarray(x1, np.float32), np.asarray(x2, np.float32),
        np.asarray(q1_w, np.float32), np.asarray(kv1_w, np.float32),
        np.asarray(q2_w, np.float32), np.asarray(kv2_w, np.float32),
        np.asarray(p1_w, np.float32), np.asarray(p1_b, np.float32),
        np.asarray(p2_w, np.float32), np.asarray(p2_b, np.float32),
        np.asarray(g1, np.float32), np.asarray(b1, np.float32),
        np.asarray(g2, np.float32), np.asarray(b2, np.float32))

    nc = _get_nc()
    res = run_bass_kernel_spmd(nc, in_maps, list(range(8)))

    def unpack(o):  # [P, CC, 9216] -> [C, 96, 96]
        return o.transpose(1, 0, 2).reshape(C, 96, 96)

    y1 = np.stack([unpack(res.results[b * 2]["out"]) for b in range(B)])
    y2 = np.stack([unpack(res.results[b * 2 + 1]["out"]) for b in range(B)])
    return y1, y2
